# revision 2
# baseline (speedup 1.0000x reference)
"""FBGCN layer on 8 Trainium2 NeuronCores — v4.

Math (reference):
    Lhp = (d_inv @ lap) @ d_inv
    Hh  = Lhp @ relu(x @ W_high)
    Hl  = GCNConv(x, edge_index, W_conv, b_conv)
    out = aL * Hl + aH * Hh

v4 vs v3 (trace-driven):
  * Each AllGather is split into TWO half-gathers (m-tiles {0,1} and
    {2,3}).  The 2MB mesh gather is bandwidth-bound (~20-30us); halving
    the payload lets the first half's flight overlap the producer's
    second half (B/D m-group-major) and the consumer (D/E) start on the
    first half's chunks while the second half is still in flight.
  * B is m-group-major (needs full R, which A produces first anyway):
    psB[0,1] complete at B-midpoint -> AG1a doorbell fires ~7us earlier.
  * D processes readback half1 for all m, then half2 for m{0,1} ->
    stage+AG2a, then half2 for m{2,3} -> stage+AG2b.
  * E processes half1 chunks as they land, then half2.
  * Staging copies moved from Vector to Scalar (v3's copies queued
    behind 20+ fp8 CASTs in the DVE FIFO, delaying the AG1 doorbell by
    ~9us).
  * Bulk loads in fewer, bigger DMAs (trigger issue is ~0.6us each on
    the sync queue; v3 spent ~8us just issuing triggers).
  * C (fp8, plain rate) is split per m-tile and placed to plug the
    AG flight gaps: C0/C1 after A_xw (AG1 flight), C2 before D-half2,
    C3 during AG2a flight.
"""

import numpy as np
import ml_dtypes

import concourse.bass as bass
import concourse.mybir as mybir
import concourse.tile as tile
from concourse import bacc
from concourse.bass_utils import run_bass_kernel_spmd

N = 4096
D = 256
E = 131072
NCORES = 8
RPC = N // NCORES          # rows per core = 512
KC = N // 128              # contraction chunks = 32
MT = RPC // 128            # output row tiles per core = 4
P = 128

BF16 = mybir.dt.bfloat16
F32 = mybir.dt.float32
FP8 = mybir.dt.float8e4
nbf16 = ml_dtypes.bfloat16
nfp8 = ml_dtypes.float8_e4m3

RELU = mybir.ActivationFunctionType.Relu
COPY = mybir.ActivationFunctionType.Copy


def slot_chunk(h, s):
    """Global chunk index for slot s (0..15) of half-gather h (0/1).

    Half h gathers m-tiles {2h, 2h+1} of every rank; the sub-AG output
    is rank-major, so slot s = (rank r = s//2, tile t = s%2) = global
    chunk 4r + 2h + t."""
    return 4 * (s // 2) + 2 * h + (s % 2)


def build_program(repeat: int = 1, ablate: frozenset = frozenset(), serial: bool = True):
    """Build the SPMD per-core program (identical on all cores)."""
    nc = bacc.Bacc(num_devices=NCORES)

    # ---- I/O ----  (matrix inputs come host-pre-transposed to [P, kc*m])
    xT = nc.declare_dram_parameter("xT", [P, 2 * N], BF16, isOutput=False)
    Whc = nc.declare_dram_parameter("Whc", [P, 2 * 2 * D], BF16, isOutput=False)
    dT = nc.declare_dram_parameter("dT", [P, KC * RPC], BF16, isOutput=False)
    lT = nc.declare_dram_parameter("lT", [P, KC * RPC], BF16, isOutput=False)
    aT = nc.declare_dram_parameter("aT", [P, KC * RPC], FP8, isOutput=False)
    bL = nc.declare_dram_parameter("bL", [P, D], F32, isOutput=False)
    out = nc.declare_dram_parameter("out", [RPC, D], BF16, isOutput=True)

    # collective bounce buffers: 2 half-gathers per stage, each [P, 2*D]
    cc_in = {}
    cc_out = {}
    for g in (1, 2):
        for h in (0, 1):
            cc_in[g, h] = nc.dram_tensor(f"cc{g}{h}_in", [P, 2 * D], BF16)
            cc_out[g, h] = nc.dram_tensor(
                f"cc{g}{h}_out", [NCORES * P, 2 * D], BF16, addr_space="Shared"
            )

    dT_v = dT.rearrange("p (kc m) -> p kc m", kc=KC)
    lT_v = lT.rearrange("p (kc m) -> p kc m", kc=KC)
    aT_v = aT.rearrange("p (kc m) -> p kc m", kc=KC)
    xT_v = xT.rearrange("p (kc m) -> p kc m", kc=2)
    Whc_v = Whc.rearrange("p (kc m) -> p kc m", kc=2)
    cc_in_v = {k: v.rearrange("p (mt m) -> p mt m", mt=2) for k, v in cc_in.items()}
    # sub-AG readback: rank r partition p holds tiles {2h, 2h+1} as 2 D-cols
    cc_out_v = {
        k: v.rearrange("(rc p) (mt m) -> p rc mt m", p=P, mt=2)
        for k, v in cc_out.items()
    }
    out_v = out.rearrange("(mt p) m -> p mt m", p=P)

    replica_groups = [list(range(NCORES))]

    def allgather(g, h):
        nc.gpsimd.collective_compute(
            "AllGather",
            mybir.AluOpType.bypass,
            replica_groups=replica_groups,
            ins=[cc_in[g, h][:]],
            outs=[cc_out[g, h][:]],
        )

    with tile.TileContext(nc) as tc:
        with (
            tc.tile_pool(name="const", bufs=1) as cpool,
            tc.tile_pool(name="bigmat", bufs=1) as bigpool,
            tc.tile_pool(name="acts", bufs=1) as apool,
            tc.tile_pool(name="psum", bufs=8, space="PSUM") as pspool,
            tc.tile_pool(name="outp", bufs=2) as opool,
        ):
            for _rep in range(repeat):
                if serial and _rep > 0:
                    # full flush between iterations: slope == single-shot latency
                    tc.strict_bb_all_engine_barrier()

                # ---- bulk loads, sync ring, few big DMAs, in need-order ----
                xT_sb = cpool.tile([P, 2, N], BF16, tag="xT")
                Whc_sb = cpool.tile([P, 2, 2 * D], BF16, tag="Whc")
                bL_sb = cpool.tile([P, D], F32, tag="bL")
                d_sb = bigpool.tile([P, KC, RPC], BF16, tag="d")
                a_sb = bigpool.tile([P, KC, RPC], FP8, tag="a")
                l_sb = bigpool.tile([P, KC, RPC], BF16, tag="l")
                nc.sync.dma_start(out=Whc_sb[:], in_=Whc_v)
                # x in halves so stage A starts after ~1MB lands
                for mh in range(2):
                    s = slice(mh * (N // 2), (mh + 1) * (N // 2))
                    nc.sync.dma_start(out=xT_sb[:, :, s], in_=xT_v[:, :, s])
                if "load" not in ablate:
                    for c in range(2):
                        s = slice(c * (KC // 2), (c + 1) * (KC // 2))
                        nc.sync.dma_start(out=d_sb[:, s, :], in_=dT_v[:, s, :])
                    nc.sync.dma_start(out=a_sb[:], in_=aT_v)
                    nc.sync.dma_start(out=bL_sb[:], in_=bL[:])
                    for c in range(2):
                        s = slice(c * (KC // 2), (c + 1) * (KC // 2))
                        nc.sync.dma_start(out=l_sb[:, s, :], in_=lT_v[:, s, :])
                else:
                    nc.sync.dma_start(out=d_sb[:, :1, :64], in_=dT_v[:, :1, :64])
                    nc.sync.dma_start(out=a_sb[:, :1, :128], in_=aT_v[:, :1, :128])
                    nc.sync.dma_start(out=bL_sb[:], in_=bL[:])
                    nc.sync.dma_start(out=l_sb[:, :1, :64], in_=lT_v[:, :1, :64])

                # ---- stage A (R half): R = relu(x @ aH*W_high), bf16 ----
                R_sb = apool.tile([P, KC, D], BF16, tag="R")
                xw_sb = apool.tile([P, KC, D], FP8, tag="xw")
                if "A" in ablate:
                    nc.sync.dma_start(out=R_sb[:, :1, :64], in_=dT_v[:, :1, :64])
                    nc.sync.dma_start(out=xw_sb[:, :1, :128], in_=aT_v[:, :1, :128])
                if "A" not in ablate:
                    for m in range(KC):
                        psA = pspool.tile([P, D], F32, tag="ps", name=f"psA{m}_{_rep}")
                        for k in range(2):
                            nc.tensor.matmul(
                                out=psA[:],
                                lhsT=xT_sb[:, k, m * P:(m + 1) * P],
                                rhs=Whc_sb[:, k, :D],
                                start=(k == 0),
                                stop=(k == 1),
                            )
                        nc.scalar.activation(R_sb[:, m, :], psA[:], RELU)

                def gather_store(g, h, pst, pos):
                    # PSUM -> SBUF copy on Scalar (keeps the DVE FIFO free of
                    # staging work), then stage to the collective input.
                    t = opool.tile([P, D], BF16, tag="gst", name=f"gs{g}{h}{pos}_{_rep}")
                    nc.scalar.activation(t[:], pst[:], COPY)
                    nc.scalar.dma_start(out=cc_in_v[g, h][:, pos, :], in_=t[:])

                def gather_load(g, h, dst_sb):
                    # half h's 16 slots land at slot indices [16h, 16h+16);
                    # 1 DMA, 8KB contiguous per partition in dst
                    nc.scalar.dma_start(
                        out=dst_sb[:, 16 * h:16 * (h + 1), :].rearrange(
                            "p (rc mt) m -> p rc mt m", mt=2
                        ),
                        in_=cc_out_v[g, h][:, :, :, :],
                    )

                Hl_sb = opool.tile([P, MT, D], F32, tag="Hl")

                # ---- stage B (m-group-major): P1_loc = d_inv[rows] @ R ----
                # group (2h, 2h+1) completes at half-point -> AG1h fires early
                if "B" not in ablate:
                    psB = {}
                    for m in range(MT):
                        psB[m] = pspool.tile([P, D], F32, tag="ps", name=f"psB{m}_{_rep}")
                    for h in (0, 1):
                        for c in range(KC):
                            for m in (2 * h, 2 * h + 1):
                                nc.tensor.matmul(
                                    out=psB[m][:],
                                    lhsT=d_sb[:, c, m * P:(m + 1) * P],
                                    rhs=R_sb[:, c, :],
                                    start=(c == 0),
                                    stop=(c == KC - 1),
                                )
                        for m in (2 * h, 2 * h + 1):
                            gather_store(1, h, psB[m], m - 2 * h)
                        if "AG1" not in ablate:
                            allgather(1, h)

                # ---- stage A (xw half, deferred): xw = fp8(x @ W_conv) ----
                # covers AG1 flight
                if "A" not in ablate:
                    for m in range(KC):
                        psX = pspool.tile([P, D], F32, tag="ps", name=f"psX{m}_{_rep}")
                        for k in range(2):
                            nc.tensor.matmul(
                                out=psX[:],
                                lhsT=xT_sb[:, k, m * P:(m + 1) * P],
                                rhs=Whc_sb[:, k, D:],
                                start=(k == 0),
                                stop=(k == 1),
                            )
                        nc.vector.tensor_copy(xw_sb[:, m, :], psX[:])

                def stage_c_mtile(m):
                    # plain fp8 matmuls (same rate as bf16, half the SBUF bytes)
                    ps = pspool.tile([P, D], F32, tag="ps", name=f"psC{m}_{_rep}")
                    for c in range(KC):
                        nc.tensor.matmul(
                            out=ps[:],
                            lhsT=a_sb[:, c, m * P:(m + 1) * P],
                            rhs=xw_sb[:, c, :],
                            start=(c == 0),
                            stop=(c == KC - 1),
                        )
                    nc.vector.tensor_add(Hl_sb[:, m, :], ps[:], bL_sb[:])

                if "C" not in ablate:
                    stage_c_mtile(0)
                    stage_c_mtile(1)
                else:
                    for m in range(MT):
                        nc.vector.tensor_copy(Hl_sb[:, m, :], bL_sb[:])

                # ---- stage D: P2_loc = lap[rows] @ P1, half-pipelined ----
                # P1 slot s of half h = global chunk slot_chunk(h, s)
                P1_sb = apool.tile([P, KC, D], BF16, tag="P1")
                gather_load(1, 0, P1_sb)
                gather_load(1, 1, P1_sb)
                psD = {}
                if "D" not in ablate:
                    for m in range(MT):
                        psD[m] = pspool.tile([P, D], F32, tag="ps", name=f"psD{m}_{_rep}")
                    # half 0 chunks, all m
                    for s in range(16):
                        cg = slot_chunk(0, s)
                        for m in range(MT):
                            nc.tensor.matmul(
                                out=psD[m][:],
                                lhsT=l_sb[:, cg, m * P:(m + 1) * P],
                                rhs=P1_sb[:, s, :],
                                start=(s == 0),
                                stop=False,
                            )
                    if "C" not in ablate:
                        stage_c_mtile(2)       # fills the wait for readback h1
                    # half 1 chunks, m{0,1} -> AG2a, then m{2,3} -> AG2b
                    for mg in (0, 1):
                        for s in range(16):
                            cg = slot_chunk(1, s)
                            for m in (2 * mg, 2 * mg + 1):
                                nc.tensor.matmul(
                                    out=psD[m][:],
                                    lhsT=l_sb[:, cg, m * P:(m + 1) * P],
                                    rhs=P1_sb[:, 16 + s, :],
                                    start=False,
                                    stop=(s == 15),
                                )
                        for m in (2 * mg, 2 * mg + 1):
                            gather_store(2, mg, psD[m], m - 2 * mg)
                        if "AG2" not in ablate:
                            allgather(2, mg)

                # ---- stage C (rest): covers AG2 flight ----
                if "C" not in ablate:
                    stage_c_mtile(3)

                # ---- stage E: out = Hl + d_inv[rows] @ P2, half-pipelined ----
                P2_sb = apool.tile([P, KC, D], BF16, tag="P2")
                gather_load(2, 0, P2_sb)
                gather_load(2, 1, P2_sb)
                if "E" not in ablate:
                    psE = {}
                    for m in range(MT):
                        psE[m] = pspool.tile([P, D], F32, tag="ps", name=f"psE{m}_{_rep}")
                    for h in (0, 1):
                        for s in range(16):
                            cg = slot_chunk(h, s)
                            for m in range(MT):
                                nc.tensor.matmul(
                                    out=psE[m][:],
                                    lhsT=d_sb[:, cg, m * P:(m + 1) * P],
                                    rhs=P2_sb[:, 16 * h + s, :],
                                    start=(h == 0 and s == 0),
                                    stop=(h == 1 and s == 15),
                                )
                    for m in range(MT):
                        o_sb = opool.tile([P, D], BF16, tag="osb", name=f"os{m}_{_rep}")
                        nc.vector.tensor_add(o_sb[:], psE[m][:], Hl_sb[:, m, :])
                        nc.scalar.dma_start(out=out_v[:, m, :], in_=o_sb[:])

    nc.finalize()
    return nc


def prep_inputs(x, edge_index, lap, d_inv, W_high, W_conv, b_conv, aL, aH):
    """Host-side sharding/layout: build per-core input maps."""
    x = np.asarray(x, dtype=np.float32)
    lap = np.asarray(lap, dtype=np.float32)
    d_inv = np.asarray(d_inv, dtype=np.float32)
    W_high = np.asarray(W_high, dtype=np.float32)
    W_conv = np.asarray(W_conv, dtype=np.float32)
    b_conv = np.asarray(b_conv, dtype=np.float32)
    aLs = float(np.asarray(aL).reshape(-1)[0])
    aHs = float(np.asarray(aH).reshape(-1)[0])
    src = np.asarray(edge_index[0], dtype=np.int64)
    dst = np.asarray(edge_index[1], dtype=np.int64)

    # symmetric GCN normalization (with self-loops) folded into a dense adjacency
    deg = np.bincount(dst, minlength=N).astype(np.float32) + 1.0
    dis = 1.0 / np.sqrt(deg)
    A_T = np.zeros((N, N), dtype=np.float32)           # A_T[src, dst]
    np.add.at(A_T, (src, dst), aLs * dis[src] * dis[dst])
    A_T[np.arange(N), np.arange(N)] += aLs * dis * dis

    def to_pkm(arrT):
        # [K, M] -> [P, kc*M]: element (p, c*M + m) = arrT[128*c + p, m]
        Kdim, Mdim = arrT.shape
        kc = Kdim // P
        a = arrT.reshape(kc, P, Mdim)
        return np.ascontiguousarray(a.transpose(1, 0, 2).reshape(P, kc * Mdim))

    xT = to_pkm(np.ascontiguousarray(x.T).astype(nbf16))
    Whc = to_pkm(np.concatenate([W_high * aHs, W_conv], axis=1).astype(nbf16))
    bLb = np.broadcast_to(aLs * b_conv, (P, D)).astype(np.float32).copy()
    dT_full = np.ascontiguousarray(d_inv.T).astype(nbf16)
    lT_full = np.ascontiguousarray(lap.T).astype(nbf16)
    aT_full = np.clip(A_T, -240, 240).astype(nfp8)

    in_maps = []
    for i in range(NCORES):
        sl = slice(i * RPC, (i + 1) * RPC)
        in_maps.append({
            "xT": xT,
            "Whc": Whc,
            "dT": to_pkm(dT_full[:, sl]),
            "lT": to_pkm(lT_full[:, sl]),
            "aT": to_pkm(aT_full[:, sl]),
            "bL": bLb,
        })
    return in_maps


def kernel(x, edge_index, lap, d_inv, W_high, W_conv, b_conv, aL, aH):
    in_maps = prep_inputs(x, edge_index, lap, d_inv, W_high, W_conv, b_conv, aL, aH)
    nc = build_program()
    res = run_bass_kernel_spmd(nc, in_maps, list(range(NCORES)))
    return np.concatenate(
        [res.results[i]["out"] for i in range(NCORES)], axis=0
    ).astype(np.float32)


# revision 7
# speedup vs baseline: 1.0955x; 1.0955x over previous
"""FBGCN layer on 8 Trainium2 NeuronCores — v5.

Math (reference):
    Lhp = (d_inv @ lap) @ d_inv
    Hh  = Lhp @ relu(x @ W_high)
    Hl  = GCNConv(x, edge_index, W_conv, b_conv)
    out = aL * Hl + aH * Hh

v5 vs v3/v4 (trace-driven):
  * Collectives are ~15-25us nearly size-independent here, so v4's
    split-gather regressed (4 collectives > 2).  Back to exactly TWO
    AllGathers (structural minimum for the 3-matmul chain).
  * Staging copies on Scalar (v3 queued them behind 20+ fp8 CASTs on
    the DVE FIFO, delaying the AG1 doorbell ~10us), into ONE [P,MT*D]
    staging tile -> ONE 512KB DMA to the bounce buffer (fewer trigger
    slots, bigger transfer).
  * Stages C and E run in transposed form (out.T = V.T @ M.T): the thin
    operand becomes lhsT (natural layout, as delivered by the AG
    readback) and the wide matrix (aT / dT, host-pre-transposed) is the
    rhs with free dim 512 -> 64 matmuls instead of 128 per stage at
    ~98% PE column efficiency.  Output lands D-major [2*P, RPC]; the
    host transposes (free).  The C/E adds become per-partition bias
    (bT) + tensor_tensor in transposed space.
  * C is split into chunk ranges placed to plug both AG windows:
    chunks 0..15 after A_xw (AG1 flight), 16..31 after the AG2 doorbell.
  * Bulk loads in 8 large DMAs, need-order; out stores on scalar ring.
"""

import numpy as np
import ml_dtypes

import concourse.bass as bass
import concourse.mybir as mybir
import concourse.tile as tile
from concourse import bacc
from concourse.bass_utils import run_bass_kernel_spmd

N = 4096
D = 256
E = 131072
NCORES = 8
RPC = N // NCORES          # rows per core = 512
KC = N // 128              # contraction chunks = 32
MT = RPC // 128            # output row tiles per core = 4
P = 128

BF16 = mybir.dt.bfloat16
F32 = mybir.dt.float32
FP8 = mybir.dt.float8e4
nbf16 = ml_dtypes.bfloat16
nfp8 = ml_dtypes.float8_e4m3

RELU = mybir.ActivationFunctionType.Relu
COPY = mybir.ActivationFunctionType.Copy
IDENT = mybir.ActivationFunctionType.Identity

# readback halves: slots (= global chunks) 0..15 are ranks 0-3, 16..31 ranks 4-7
HALF1 = list(range(KC // 2))
HALF2 = list(range(KC // 2, KC))


def build_program(repeat: int = 1, ablate: frozenset = frozenset(), serial: bool = True):
    """Build the SPMD per-core program (identical on all cores)."""
    nc = bacc.Bacc(num_devices=NCORES)

    # ---- I/O ----  (matrix inputs come host-pre-transposed to [P, kc*m])
    xT = nc.declare_dram_parameter("xT", [P, 2 * N], BF16, isOutput=False)
    Whc = nc.declare_dram_parameter("Whc", [P, 2 * 2 * D], BF16, isOutput=False)
    dT = nc.declare_dram_parameter("dT", [P, KC * RPC], BF16, isOutput=False)
    lT = nc.declare_dram_parameter("lT", [P, KC * RPC], BF16, isOutput=False)
    aT = nc.declare_dram_parameter("aT", [P, KC * RPC], FP8, isOutput=False)
    bT = nc.declare_dram_parameter("bT", [P, 2], F32, isOutput=False)
    # transposed output: row = D coordinate (2 halves of 128), col = local row
    out = nc.declare_dram_parameter("out", [2 * P, RPC], BF16, isOutput=True)

    # collective bounce buffers: one gather = one collective of [P, MT*D]
    cc_in = {}
    cc_out = {}
    for g in (1, 2):
        cc_in[g] = nc.dram_tensor(f"cc{g}_in", [P, MT * D], BF16)
        cc_out[g] = nc.dram_tensor(
            f"cc{g}_out", [NCORES * P, MT * D], BF16, addr_space="Shared"
        )

    dT_v = dT.rearrange("p (kc m) -> p kc m", kc=KC)
    lT_v = lT.rearrange("p (kc m) -> p kc m", kc=KC)
    aT_v = aT.rearrange("p (kc m) -> p kc m", kc=KC)
    xT_v = xT.rearrange("p (kc m) -> p kc m", kc=2)
    Whc_v = Whc.rearrange("p (kc m) -> p kc m", kc=2)
    cc_in_v = {k: v.rearrange("p (mt m) -> p mt m", mt=MT) for k, v in cc_in.items()}
    # readback: rank r partition p holds chunks 4r..4r+3 as 4 contiguous D-cols
    cc_out_v = {
        k: v.rearrange("(rc p) (mt m) -> p rc mt m", p=P, mt=MT)
        for k, v in cc_out.items()
    }

    replica_groups = [list(range(NCORES))]

    def allgather(g):
        nc.gpsimd.collective_compute(
            "AllGather",
            mybir.AluOpType.bypass,
            replica_groups=replica_groups,
            ins=[cc_in[g][:]],
            outs=[cc_out[g][:]],
        )

    with tile.TileContext(nc) as tc:
        with (
            tc.tile_pool(name="const", bufs=1) as cpool,
            tc.tile_pool(name="bigmat", bufs=1) as bigpool,
            tc.tile_pool(name="acts", bufs=1) as apool,
            tc.tile_pool(name="psum", bufs=4, space="PSUM") as pspool,
            tc.tile_pool(name="outp", bufs=2) as opool,
        ):
            for _rep in range(repeat):
                if serial and _rep > 0:
                    # full flush between iterations: slope == single-shot latency
                    tc.strict_bb_all_engine_barrier()

                # ---- bulk loads, sync ring, few big DMAs, in need-order ----
                xT_sb = cpool.tile([P, 2, N], BF16, tag="xT")
                Whc_sb = cpool.tile([P, 2, 2 * D], BF16, tag="Whc")
                bT_sb = cpool.tile([P, 2], F32, tag="bT")
                d_sb = bigpool.tile([P, KC, RPC], BF16, tag="d")
                a_sb = bigpool.tile([P, KC, RPC], FP8, tag="a")
                l_sb = bigpool.tile([P, KC, RPC], BF16, tag="l")
                nc.sync.dma_start(out=Whc_sb[:], in_=Whc_v)
                # x in halves so stage A starts after ~1MB lands
                for mh in range(2):
                    s = slice(mh * (N // 2), (mh + 1) * (N // 2))
                    nc.sync.dma_start(out=xT_sb[:, :, s], in_=xT_v[:, :, s])
                if "load" not in ablate:
                    for c in range(2):
                        s = slice(c * (KC // 2), (c + 1) * (KC // 2))
                        nc.sync.dma_start(out=d_sb[:, s, :], in_=dT_v[:, s, :])
                    nc.sync.dma_start(out=a_sb[:], in_=aT_v)
                    nc.sync.dma_start(out=bT_sb[:], in_=bT[:])
                    for c in range(2):
                        s = slice(c * (KC // 2), (c + 1) * (KC // 2))
                        nc.sync.dma_start(out=l_sb[:, s, :], in_=lT_v[:, s, :])
                else:
                    nc.sync.dma_start(out=d_sb[:, :1, :64], in_=dT_v[:, :1, :64])
                    nc.sync.dma_start(out=a_sb[:, :1, :128], in_=aT_v[:, :1, :128])
                    nc.sync.dma_start(out=bT_sb[:], in_=bT[:])
                    nc.sync.dma_start(out=l_sb[:, :1, :64], in_=lT_v[:, :1, :64])

                # ---- stage A (R half): R = relu(x @ aH*W_high), bf16 ----
                R_sb = apool.tile([P, KC, D], BF16, tag="R")
                xw_sb = apool.tile([P, KC, D], FP8, tag="xw")
                if "A" in ablate:
                    nc.sync.dma_start(out=R_sb[:, :1, :64], in_=dT_v[:, :1, :64])
                    nc.sync.dma_start(out=xw_sb[:, :1, :128], in_=aT_v[:, :1, :128])
                if "A" not in ablate:
                    for m in range(KC):
                        psA = pspool.tile([P, D], F32, tag="ps", name=f"psA{m}_{_rep}")
                        for k in range(2):
                            nc.tensor.matmul(
                                out=psA[:],
                                lhsT=xT_sb[:, k, m * P:(m + 1) * P],
                                rhs=Whc_sb[:, k, :D],
                                start=(k == 0),
                                stop=(k == 1),
                            )
                        nc.scalar.activation(R_sb[:, m, :], psA[:], RELU)

                def gather_store(g, psts):
                    # PSUM -> one SBUF staging tile on Scalar, then ONE 512KB DMA
                    t = opool.tile([P, MT, D], BF16, tag="gst", name=f"gs{g}_{_rep}")
                    for m in range(MT):
                        nc.scalar.activation(t[:, m, :], psts[m][:], COPY)
                    nc.scalar.dma_start(out=cc_in_v[g][:, :, :], in_=t[:])

                def gather_load(g, half, dst_sb):
                    # rank half -> 16 chunk slots; 2KB contiguous per partition
                    sl = slice(0, KC // 2) if half == 1 else slice(KC // 2, KC)
                    rc = slice(0, NCORES // 2) if half == 1 else slice(NCORES // 2, NCORES)
                    nc.scalar.dma_start(
                        out=dst_sb[:, sl, :].rearrange("p (rc mt) m -> p rc mt m", mt=MT),
                        in_=cc_out_v[g][:, rc, :, :],
                    )

                # ---- stage B (chunk-major): P1_loc = d_inv[rows] @ R ----
                if "B" not in ablate:
                    psB = {}
                    for m in range(MT):
                        psB[m] = pspool.tile([P, D], F32, tag="ps", name=f"psB{m}_{_rep}")
                    for c in range(KC):
                        for m in range(MT):
                            nc.tensor.matmul(
                                out=psB[m][:],
                                lhsT=d_sb[:, c, m * P:(m + 1) * P],
                                rhs=R_sb[:, c, :],
                                start=(c == 0),
                                stop=(c == KC - 1),
                            )
                    gather_store(1, psB)
                    if "AG1" not in ablate:
                        allgather(1)

                # ---- stage A (xw half, deferred): xw = fp8(x @ W_conv) ----
                # covers AG1 flight; C chunks 0..15 cover the readback
                if "A" not in ablate:
                    for m in range(KC):
                        psX = pspool.tile([P, D], F32, tag="ps", name=f"psX{m}_{_rep}")
                        for k in range(2):
                            nc.tensor.matmul(
                                out=psX[:],
                                lhsT=xT_sb[:, k, m * P:(m + 1) * P],
                                rhs=Whc_sb[:, k, D:],
                                start=(k == 0),
                                stop=(k == 1),
                            )
                        nc.vector.tensor_copy(xw_sb[:, m, :], psX[:])

                # ---- stage C (transposed): HlT = (A_T.T @ xw).T accumulation
                # psC[h] [P,512] += xw[:,c,h*128:].T @ aT[:,c,:]  (fp8, N=512)
                psC = {}
                if "C" not in ablate:
                    for h in range(2):
                        psC[h] = pspool.tile(
                            [P, RPC], F32, tag="psw", bufs=4, name=f"psC{h}_{_rep}"
                        )

                def stage_c_chunks(cs, first, last):
                    for c in cs:
                        for h in range(2):
                            nc.tensor.matmul(
                                out=psC[h][:],
                                lhsT=xw_sb[:, c, h * P:(h + 1) * P],
                                rhs=a_sb[:, c, :],
                                start=(first and c == cs[0]),
                                stop=(last and c == cs[-1]),
                            )

                if "C" not in ablate:
                    stage_c_chunks(list(range(16)), True, False)

                # ---- stage D: P2_loc = lap[rows] @ P1, split on rb halves ----
                P1_sb = apool.tile([P, KC, D], BF16, tag="P1")
                gather_load(1, 1, P1_sb)
                gather_load(1, 2, P1_sb)
                psD = {}
                if "D" not in ablate:
                    for m in range(MT):
                        psD[m] = pspool.tile([P, D], F32, tag="ps", name=f"psD{m}_{_rep}")
                    for half in (HALF1, HALF2):
                        for m in range(MT):
                            for i, c in enumerate(half):
                                nc.tensor.matmul(
                                    out=psD[m][:],
                                    lhsT=l_sb[:, c, m * P:(m + 1) * P],
                                    rhs=P1_sb[:, c, :],
                                    start=(half is HALF1 and i == 0),
                                    stop=(half is HALF2 and i == len(half) - 1),
                                )
                    gather_store(2, psD)
                    if "AG2" not in ablate:
                        allgather(2)

                # ---- stage C (rest): covers AG2 flight; fold bias on evacuate
                HlT_sb = opool.tile([P, 2, RPC], BF16, tag="HlT")
                if "C" not in ablate:
                    stage_c_chunks(list(range(16, KC)), False, True)
                    for h in range(2):
                        nc.scalar.activation(
                            HlT_sb[:, h, :], psC[h][:], IDENT, bias=bT_sb[:, h:h + 1]
                        )
                else:
                    for h in range(2):
                        nc.vector.memset(HlT_sb[:, h, :], 0.0)

                # ---- stage E (transposed): HhT[h] += P2[:,c,h].T @ dT[:,c,:]
                P2_sb = apool.tile([P, KC, D], BF16, tag="P2")
                gather_load(2, 1, P2_sb)
                gather_load(2, 2, P2_sb)
                if "E" not in ablate:
                    psE = {}
                    for h in range(2):
                        psE[h] = pspool.tile(
                            [P, RPC], F32, tag="psw", bufs=4, name=f"psE{h}_{_rep}"
                        )
                    for half in (HALF1, HALF2):
                        for i, c in enumerate(half):
                            for h in range(2):
                                nc.tensor.matmul(
                                    out=psE[h][:],
                                    lhsT=P2_sb[:, c, h * P:(h + 1) * P],
                                    rhs=d_sb[:, c, :],
                                    start=(half is HALF1 and i == 0),
                                    stop=(half is HALF2 and i == len(half) - 1),
                                )
                    for h in range(2):
                        o_sb = opool.tile([P, RPC], BF16, tag="osb", name=f"os{h}_{_rep}")
                        nc.vector.tensor_add(o_sb[:], psE[h][:], HlT_sb[:, h, :])
                        nc.scalar.dma_start(out=out[h * P:(h + 1) * P, :], in_=o_sb[:])

    nc.finalize()
    return nc


def prep_inputs(x, edge_index, lap, d_inv, W_high, W_conv, b_conv, aL, aH):
    """Host-side sharding/layout: build per-core input maps."""
    x = np.asarray(x, dtype=np.float32)
    lap = np.asarray(lap, dtype=np.float32)
    d_inv = np.asarray(d_inv, dtype=np.float32)
    W_high = np.asarray(W_high, dtype=np.float32)
    W_conv = np.asarray(W_conv, dtype=np.float32)
    b_conv = np.asarray(b_conv, dtype=np.float32)
    aLs = float(np.asarray(aL).reshape(-1)[0])
    aHs = float(np.asarray(aH).reshape(-1)[0])
    src = np.asarray(edge_index[0], dtype=np.int64)
    dst = np.asarray(edge_index[1], dtype=np.int64)

    # symmetric GCN normalization (with self-loops) folded into a dense adjacency
    deg = np.bincount(dst, minlength=N).astype(np.float32) + 1.0
    dis = 1.0 / np.sqrt(deg)
    A_T = np.zeros((N, N), dtype=np.float32)           # A_T[src, dst]
    np.add.at(A_T, (src, dst), aLs * dis[src] * dis[dst])
    A_T[np.arange(N), np.arange(N)] += aLs * dis * dis

    def to_pkm(arrT):
        # [K, M] -> [P, kc*M]: element (p, c*M + m) = arrT[128*c + p, m]
        Kdim, Mdim = arrT.shape
        kc = Kdim // P
        a = arrT.reshape(kc, P, Mdim)
        return np.ascontiguousarray(a.transpose(1, 0, 2).reshape(P, kc * Mdim))

    xT = to_pkm(np.ascontiguousarray(x.T).astype(nbf16))
    Whc = to_pkm(np.concatenate([W_high * aHs, W_conv], axis=1).astype(nbf16))
    # bias along D (partition axis in transposed space): [P, 2] f32
    bTm = np.ascontiguousarray((aLs * b_conv).reshape(2, P).T).astype(np.float32)
    dT_full = np.ascontiguousarray(d_inv.T).astype(nbf16)
    lT_full = np.ascontiguousarray(lap.T).astype(nbf16)
    aT_full = np.clip(A_T, -240, 240).astype(nfp8)

    in_maps = []
    for i in range(NCORES):
        sl = slice(i * RPC, (i + 1) * RPC)
        in_maps.append({
            "xT": xT,
            "Whc": Whc,
            "dT": to_pkm(dT_full[:, sl]),
            "lT": to_pkm(lT_full[:, sl]),
            "aT": to_pkm(aT_full[:, sl]),
            "bT": bTm,
        })
    return in_maps


def kernel(x, edge_index, lap, d_inv, W_high, W_conv, b_conv, aL, aH):
    in_maps = prep_inputs(x, edge_index, lap, d_inv, W_high, W_conv, b_conv, aL, aH)
    nc = build_program()
    res = run_bass_kernel_spmd(nc, in_maps, list(range(NCORES)))
    # per-core output is D-major [256, 512]; transpose back to [512, 256]
    return np.concatenate(
        [res.results[i]["out"].T for i in range(NCORES)], axis=0
    ).astype(np.float32)


# revision 13
# speedup vs baseline: 1.1406x; 1.0412x over previous
"""FBGCN layer on 8 Trainium2 NeuronCores — v5.

Math (reference):
    Lhp = (d_inv @ lap) @ d_inv
    Hh  = Lhp @ relu(x @ W_high)
    Hl  = GCNConv(x, edge_index, W_conv, b_conv)
    out = aL * Hl + aH * Hh

v5 vs v3/v4 (trace-driven):
  * Collectives are ~15-25us nearly size-independent here, so v4's
    split-gather regressed (4 collectives > 2).  Back to exactly TWO
    AllGathers (structural minimum for the 3-matmul chain).
  * Staging copies on Scalar (v3 queued them behind 20+ fp8 CASTs on
    the DVE FIFO, delaying the AG1 doorbell ~10us), into ONE [P,MT*D]
    staging tile -> ONE 512KB DMA to the bounce buffer (fewer trigger
    slots, bigger transfer).
  * Stages C and E run in transposed form (out.T = V.T @ M.T): the thin
    operand becomes lhsT (natural layout, as delivered by the AG
    readback) and the wide matrix (aT / dT, host-pre-transposed) is the
    rhs with free dim 512 -> 64 matmuls instead of 128 per stage at
    ~98% PE column efficiency.  Output lands D-major [2*P, RPC]; the
    host transposes (free).  The C/E adds become per-partition bias
    (bT) + tensor_tensor in transposed space.
  * C is split into chunk ranges placed to plug both AG windows:
    chunks 0..15 after A_xw (AG1 flight), 16..31 after the AG2 doorbell.
  * Bulk loads in 8 large DMAs, need-order; out stores on scalar ring.
"""

import numpy as np
import ml_dtypes

import concourse.bass as bass
import concourse.mybir as mybir
import concourse.tile as tile
from concourse import bacc
from concourse.bass_utils import run_bass_kernel_spmd

N = 4096
D = 256
E = 131072
NCORES = 8
RPC = N // NCORES          # rows per core = 512
KC = N // 128              # contraction chunks = 32
MT = RPC // 128            # output row tiles per core = 4
P = 128

BF16 = mybir.dt.bfloat16
F32 = mybir.dt.float32
FP8 = mybir.dt.float8e4
nbf16 = ml_dtypes.bfloat16
nfp8 = ml_dtypes.float8_e4m3

RELU = mybir.ActivationFunctionType.Relu
COPY = mybir.ActivationFunctionType.Copy
IDENT = mybir.ActivationFunctionType.Identity

# readback halves: slots (= global chunks) 0..15 are ranks 0-3, 16..31 ranks 4-7
HALF1 = list(range(KC // 2))
HALF2 = list(range(KC // 2, KC))


def build_program(repeat: int = 1, ablate: frozenset = frozenset(), serial: bool = True):
    """Build the SPMD per-core program (identical on all cores)."""
    nc = bacc.Bacc(num_devices=NCORES)

    # ---- I/O ----  (matrix inputs come host-pre-transposed to [P, kc*m])
    xT = nc.declare_dram_parameter("xT", [P, 2 * N], BF16, isOutput=False)
    Whc = nc.declare_dram_parameter("Whc", [P, 2 * 2 * D], BF16, isOutput=False)
    dT = nc.declare_dram_parameter("dT", [P, KC * RPC], BF16, isOutput=False)
    lT = nc.declare_dram_parameter("lT", [P, KC * RPC], BF16, isOutput=False)
    aT = nc.declare_dram_parameter("aT", [P, KC * RPC], FP8, isOutput=False)
    bT = nc.declare_dram_parameter("bT", [P, 2], F32, isOutput=False)
    # transposed output: row = D coordinate (2 halves of 128), col = local row
    out = nc.declare_dram_parameter("out", [2 * P, RPC], BF16, isOutput=True)

    # collective bounce buffers: one gather = one collective of [P, MT*D]
    cc_in = {}
    cc_out = {}
    for g in (1, 2):
        cc_in[g] = nc.dram_tensor(f"cc{g}_in", [P, MT * D], BF16)
        cc_out[g] = nc.dram_tensor(
            f"cc{g}_out", [NCORES * P, MT * D], BF16, addr_space="Shared"
        )

    dT_v = dT.rearrange("p (kc m) -> p kc m", kc=KC)
    lT_v = lT.rearrange("p (kc m) -> p kc m", kc=KC)
    aT_v = aT.rearrange("p (kc m) -> p kc m", kc=KC)
    xT_v = xT.rearrange("p (kc m) -> p kc m", kc=2)
    Whc_v = Whc.rearrange("p (kc m) -> p kc m", kc=2)
    cc_in_v = {k: v.rearrange("p (mt m) -> p mt m", mt=MT) for k, v in cc_in.items()}
    # readback: rank r partition p holds chunks 4r..4r+3 as 4 contiguous D-cols
    cc_out_v = {
        k: v.rearrange("(rc p) (mt m) -> p rc mt m", p=P, mt=MT)
        for k, v in cc_out.items()
    }

    replica_groups = [list(range(NCORES))]

    def allgather(g):
        nc.gpsimd.collective_compute(
            "AllGather",
            mybir.AluOpType.bypass,
            replica_groups=replica_groups,
            ins=[cc_in[g][:]],
            outs=[cc_out[g][:]],
        )

    with tile.TileContext(nc) as tc:
        with (
            tc.tile_pool(name="const", bufs=1) as cpool,
            tc.tile_pool(name="bigmat", bufs=1) as bigpool,
            tc.tile_pool(name="acts", bufs=1) as apool,
            tc.tile_pool(name="psum", bufs=4, space="PSUM") as pspool,
            tc.tile_pool(name="outp", bufs=2) as opool,
        ):
            for _rep in range(repeat):
                if serial and _rep > 0:
                    # full flush between iterations: slope == single-shot latency
                    tc.strict_bb_all_engine_barrier()

                # ---- bulk loads, sync ring, few big DMAs, in need-order ----
                xT_sb = cpool.tile([P, 2, N], BF16, tag="xT")
                Whc_sb = cpool.tile([P, 2, 2 * D], BF16, tag="Whc")
                bT_sb = cpool.tile([P, 2], F32, tag="bT")
                d_sb = bigpool.tile([P, KC, RPC], BF16, tag="d")
                a_sb = bigpool.tile([P, KC, RPC], FP8, tag="a")
                l_sb = bigpool.tile([P, KC, RPC], BF16, tag="l")
                nc.sync.dma_start(out=Whc_sb[:], in_=Whc_v)
                # x in halves so stage A starts after ~1MB lands
                for mh in range(2):
                    s = slice(mh * (N // 2), (mh + 1) * (N // 2))
                    nc.sync.dma_start(out=xT_sb[:, :, s], in_=xT_v[:, :, s])
                if "load" not in ablate:
                    for c in range(2):
                        s = slice(c * (KC // 2), (c + 1) * (KC // 2))
                        nc.sync.dma_start(out=d_sb[:, s, :], in_=dT_v[:, s, :])
                    nc.sync.dma_start(out=a_sb[:], in_=aT_v)
                    nc.sync.dma_start(out=bT_sb[:], in_=bT[:])
                    for c in range(2):
                        s = slice(c * (KC // 2), (c + 1) * (KC // 2))
                        nc.sync.dma_start(out=l_sb[:, s, :], in_=lT_v[:, s, :])
                else:
                    nc.sync.dma_start(out=d_sb[:, :1, :64], in_=dT_v[:, :1, :64])
                    nc.sync.dma_start(out=a_sb[:, :1, :128], in_=aT_v[:, :1, :128])
                    nc.sync.dma_start(out=bT_sb[:], in_=bT[:])
                    nc.sync.dma_start(out=l_sb[:, :1, :64], in_=lT_v[:, :1, :64])

                # ---- stage A (R half): R = relu(x @ aH*W_high), bf16 ----
                R_sb = apool.tile([P, KC, D], BF16, tag="R")
                xw_sb = apool.tile([P, KC, D], FP8, tag="xw")
                if "A" in ablate:
                    nc.sync.dma_start(out=R_sb[:, :1, :64], in_=dT_v[:, :1, :64])
                    nc.sync.dma_start(out=xw_sb[:, :1, :128], in_=aT_v[:, :1, :128])
                if "A" not in ablate:
                    for m in range(KC):
                        psA = pspool.tile([P, D], F32, tag="ps", name=f"psA{m}_{_rep}")
                        for k in range(2):
                            nc.tensor.matmul(
                                out=psA[:],
                                lhsT=xT_sb[:, k, m * P:(m + 1) * P],
                                rhs=Whc_sb[:, k, :D],
                                start=(k == 0),
                                stop=(k == 1),
                            )
                        # alternate relu between Scalar (activation) and Vector
                        # (max with 0) so neither engine's ~0.37us/chunk
                        # serializes the A->B feed
                        if m % 2 == 0:
                            nc.scalar.activation(R_sb[:, m, :], psA[:], RELU)
                        else:
                            nc.vector.tensor_scalar_max(R_sb[:, m, :], psA[:], 0.0)

                def gather_store(g, psts):
                    # PSUM -> SBUF staging on Scalar, per-tile DMA so the first
                    # transfer starts while later tiles are still being copied
                    t = opool.tile([P, MT, D], BF16, tag="gst", name=f"gs{g}_{_rep}")
                    for m in range(MT):
                        nc.scalar.activation(t[:, m, :], psts[m][:], COPY)
                        nc.scalar.dma_start(out=cc_in_v[g][:, m, :], in_=t[:, m, :])
                    return t

                def gather_load(g, half, dst_sb):
                    # first piece small (4 chunks) so the consumer starts early,
                    # then the remaining 12 chunks of the half
                    base = 0 if half == 1 else KC // 2
                    rc0 = 0 if half == 1 else NCORES // 2
                    for (c0, c1) in ((0, 4), (4, 16)):
                        nc.scalar.dma_start(
                            out=dst_sb[:, base + c0:base + c1, :].rearrange(
                                "p (rc mt) m -> p rc mt m", mt=MT
                            ),
                            in_=cc_out_v[g][:, rc0 + c0 // MT:rc0 + c1 // MT, :, :],
                        )

                # ---- stage B (chunk-major): P1_loc = d_inv[rows] @ R ----
                if "B" not in ablate:
                    psB = {}
                    for m in range(MT):
                        psB[m] = pspool.tile([P, D], F32, tag="ps", name=f"psB{m}_{_rep}")
                    for c in range(KC):
                        for m in range(MT):
                            nc.tensor.matmul(
                                out=psB[m][:],
                                lhsT=d_sb[:, c, m * P:(m + 1) * P],
                                rhs=R_sb[:, c, :],
                                start=(c == 0),
                                stop=(c == KC - 1),
                            )
                    gather_store(1, psB)
                    if "AG1" not in ablate:
                        allgather(1)

                # ---- stage A (xw half, deferred): xw = fp8(x @ W_conv) ----
                # covers AG1 flight; C chunks 0..15 cover the readback
                if "A" not in ablate:
                    for m in range(KC):
                        psX = pspool.tile([P, D], F32, tag="ps", name=f"psX{m}_{_rep}")
                        for k in range(2):
                            nc.tensor.matmul(
                                out=psX[:],
                                lhsT=xT_sb[:, k, m * P:(m + 1) * P],
                                rhs=Whc_sb[:, k, D:],
                                start=(k == 0),
                                stop=(k == 1),
                            )
                        nc.vector.tensor_copy(xw_sb[:, m, :], psX[:])

                # ---- stage C (transposed): HlT = (A_T.T @ xw).T accumulation
                # psC[h] [P,512] += xw[:,c,h*128:].T @ aT[:,c,:]  (fp8, N=512)
                psC = {}
                if "C" not in ablate:
                    for h in range(2):
                        psC[h] = pspool.tile(
                            [P, RPC], F32, tag="psw", bufs=4, name=f"psC{h}_{_rep}"
                        )

                def stage_c_chunks(cs, first, last, lhs=None, off=0):
                    src = xw_sb if lhs is None else lhs
                    for c in cs:
                        for h in range(2):
                            nc.tensor.matmul(
                                out=psC[h][:],
                                lhsT=src[:, c - off, h * P:(h + 1) * P],
                                rhs=a_sb[:, c, :],
                                start=(first and c == cs[0]),
                                stop=(last and c == cs[-1]),
                            )

                if "C" not in ablate:
                    stage_c_chunks(list(range(16)), True, False)

                # ---- stage D: P2_loc = lap[rows] @ P1, split on rb halves ----
                P1_sb = apool.tile([P, KC, D], BF16, tag="P1")
                gather_load(1, 1, P1_sb)
                gather_load(1, 2, P1_sb)
                psD = {}
                if "D" not in ablate:
                    for m in range(MT):
                        psD[m] = pspool.tile([P, D], F32, tag="ps", name=f"psD{m}_{_rep}")
                    for half in (HALF1, HALF2):
                        for m in range(MT):
                            for i, c in enumerate(half):
                                nc.tensor.matmul(
                                    out=psD[m][:],
                                    lhsT=l_sb[:, c, m * P:(m + 1) * P],
                                    rhs=P1_sb[:, c, :],
                                    start=(half is HALF1 and i == 0),
                                    stop=(half is HALF2 and i == len(half) - 1),
                                )
                    t2 = gather_store(2, psD)
                    if "AG2" not in ablate:
                        allgather(2)

                # ---- stage C (rest): covers AG2 flight; fold bias on evacuate
                # fence: xw2 = xw[16:] + 0, where the 0 is derived from the AG2
                # staging tile -- a data dependency that stops Tile from
                # hoisting these matmuls ahead of the AG2 doorbell (v5 ran all
                # of C early, leaving the AG2 flight uncovered and E cold)
                HlT_sb = opool.tile([P, 2, RPC], BF16, tag="HlT")
                if "C" not in ablate and "D" not in ablate:
                    fence_t = opool.tile([P, 1], F32, tag="fence")
                    nc.vector.tensor_scalar_mul(fence_t[:], t2[:, MT - 1, :1], 0.0)
                    xw2_sb = apool.tile([P, KC // 2, D], FP8, tag="xw2")
                    nc.vector.tensor_scalar_add(
                        xw2_sb[:], xw_sb[:, KC // 2:, :], fence_t[:]
                    )
                if "C" not in ablate:
                    fl = xw2_sb if "D" not in ablate else None
                    stage_c_chunks(list(range(16, KC)), False, True,
                                   lhs=fl, off=0 if fl is None else 16)
                    for h in range(2):
                        nc.scalar.activation(
                            HlT_sb[:, h, :], psC[h][:], IDENT, bias=bT_sb[:, h:h + 1]
                        )
                else:
                    for h in range(2):
                        nc.vector.memset(HlT_sb[:, h, :], 0.0)

                # ---- stage E (transposed): HhT[h] += P2[:,c,h].T @ dT[:,c,:]
                P2_sb = apool.tile([P, KC, D], BF16, tag="P2")
                gather_load(2, 1, P2_sb)
                gather_load(2, 2, P2_sb)
                if "E" not in ablate:
                    psE = {}
                    for h in range(2):
                        psE[h] = pspool.tile(
                            [P, RPC], F32, tag="psw", bufs=4, name=f"psE{h}_{_rep}"
                        )
                    for half in (HALF1, HALF2):
                        for i, c in enumerate(half):
                            for h in range(2):
                                nc.tensor.matmul(
                                    out=psE[h][:],
                                    lhsT=P2_sb[:, c, h * P:(h + 1) * P],
                                    rhs=d_sb[:, c, :],
                                    start=(half is HALF1 and i == 0),
                                    stop=(half is HALF2 and i == len(half) - 1),
                                )
                    for h in range(2):
                        o_sb = opool.tile([P, RPC], BF16, tag="osb", name=f"os{h}_{_rep}")
                        nc.vector.tensor_add(o_sb[:], psE[h][:], HlT_sb[:, h, :])
                        nc.scalar.dma_start(out=out[h * P:(h + 1) * P, :], in_=o_sb[:])

    nc.finalize()
    return nc


def prep_inputs(x, edge_index, lap, d_inv, W_high, W_conv, b_conv, aL, aH):
    """Host-side sharding/layout: build per-core input maps."""
    x = np.asarray(x, dtype=np.float32)
    lap = np.asarray(lap, dtype=np.float32)
    d_inv = np.asarray(d_inv, dtype=np.float32)
    W_high = np.asarray(W_high, dtype=np.float32)
    W_conv = np.asarray(W_conv, dtype=np.float32)
    b_conv = np.asarray(b_conv, dtype=np.float32)
    aLs = float(np.asarray(aL).reshape(-1)[0])
    aHs = float(np.asarray(aH).reshape(-1)[0])
    src = np.asarray(edge_index[0], dtype=np.int64)
    dst = np.asarray(edge_index[1], dtype=np.int64)

    # symmetric GCN normalization (with self-loops) folded into a dense adjacency
    deg = np.bincount(dst, minlength=N).astype(np.float32) + 1.0
    dis = 1.0 / np.sqrt(deg)
    A_T = np.zeros((N, N), dtype=np.float32)           # A_T[src, dst]
    np.add.at(A_T, (src, dst), aLs * dis[src] * dis[dst])
    A_T[np.arange(N), np.arange(N)] += aLs * dis * dis

    def to_pkm(arrT):
        # [K, M] -> [P, kc*M]: element (p, c*M + m) = arrT[128*c + p, m]
        Kdim, Mdim = arrT.shape
        kc = Kdim // P
        a = arrT.reshape(kc, P, Mdim)
        return np.ascontiguousarray(a.transpose(1, 0, 2).reshape(P, kc * Mdim))

    xT = to_pkm(np.ascontiguousarray(x.T).astype(nbf16))
    Whc = to_pkm(np.concatenate([W_high * aHs, W_conv], axis=1).astype(nbf16))
    # bias along D (partition axis in transposed space): [P, 2] f32
    bTm = np.ascontiguousarray((aLs * b_conv).reshape(2, P).T).astype(np.float32)
    dT_full = np.ascontiguousarray(d_inv.T).astype(nbf16)
    lT_full = np.ascontiguousarray(lap.T).astype(nbf16)
    aT_full = np.clip(A_T, -240, 240).astype(nfp8)

    in_maps = []
    for i in range(NCORES):
        sl = slice(i * RPC, (i + 1) * RPC)
        in_maps.append({
            "xT": xT,
            "Whc": Whc,
            "dT": to_pkm(dT_full[:, sl]),
            "lT": to_pkm(lT_full[:, sl]),
            "aT": to_pkm(aT_full[:, sl]),
            "bT": bTm,
        })
    return in_maps


def kernel(x, edge_index, lap, d_inv, W_high, W_conv, b_conv, aL, aH):
    in_maps = prep_inputs(x, edge_index, lap, d_inv, W_high, W_conv, b_conv, aL, aH)
    nc = build_program()
    res = run_bass_kernel_spmd(nc, in_maps, list(range(NCORES)))
    # per-core output is D-major [256, 512]; transpose back to [512, 256]
    return np.concatenate(
        [res.results[i]["out"].T for i in range(NCORES)], axis=0
    ).astype(np.float32)


# revision 17
# speedup vs baseline: 1.1442x; 1.0031x over previous
"""FBGCN layer on 8 Trainium2 NeuronCores — v5.

Math (reference):
    Lhp = (d_inv @ lap) @ d_inv
    Hh  = Lhp @ relu(x @ W_high)
    Hl  = GCNConv(x, edge_index, W_conv, b_conv)
    out = aL * Hl + aH * Hh

v5 vs v3/v4 (trace-driven):
  * Collectives are ~15-25us nearly size-independent here, so v4's
    split-gather regressed (4 collectives > 2).  Back to exactly TWO
    AllGathers (structural minimum for the 3-matmul chain).
  * Staging copies on Scalar (v3 queued them behind 20+ fp8 CASTs on
    the DVE FIFO, delaying the AG1 doorbell ~10us), into ONE [P,MT*D]
    staging tile -> ONE 512KB DMA to the bounce buffer (fewer trigger
    slots, bigger transfer).
  * Stages C and E run in transposed form (out.T = V.T @ M.T): the thin
    operand becomes lhsT (natural layout, as delivered by the AG
    readback) and the wide matrix (aT / dT, host-pre-transposed) is the
    rhs with free dim 512 -> 64 matmuls instead of 128 per stage at
    ~98% PE column efficiency.  Output lands D-major [2*P, RPC]; the
    host transposes (free).  The C/E adds become per-partition bias
    (bT) + tensor_tensor in transposed space.
  * C is split into chunk ranges placed to plug both AG windows:
    chunks 0..15 after A_xw (AG1 flight), 16..31 after the AG2 doorbell.
  * Bulk loads in 8 large DMAs, need-order; out stores on scalar ring.
"""

import numpy as np
import ml_dtypes

import concourse.bass as bass
import concourse.mybir as mybir
import concourse.tile as tile
from concourse import bacc
from concourse.bass_utils import run_bass_kernel_spmd

N = 4096
D = 256
E = 131072
NCORES = 8
RPC = N // NCORES          # rows per core = 512
KC = N // 128              # contraction chunks = 32
MT = RPC // 128            # output row tiles per core = 4
P = 128

BF16 = mybir.dt.bfloat16
F32 = mybir.dt.float32
FP8 = mybir.dt.float8e4
nbf16 = ml_dtypes.bfloat16
nfp8 = ml_dtypes.float8_e4m3

RELU = mybir.ActivationFunctionType.Relu
COPY = mybir.ActivationFunctionType.Copy
IDENT = mybir.ActivationFunctionType.Identity

# readback halves: slots (= global chunks) 0..15 are ranks 0-3, 16..31 ranks 4-7
HALF1 = list(range(KC // 2))
HALF2 = list(range(KC // 2, KC))


def build_program(repeat: int = 1, ablate: frozenset = frozenset(), serial: bool = True):
    """Build the SPMD per-core program (identical on all cores)."""
    nc = bacc.Bacc(num_devices=NCORES)

    # ---- I/O ----  (matrix inputs come host-pre-transposed to [P, kc*m])
    xT = nc.declare_dram_parameter("xT", [P, 2 * N], BF16, isOutput=False)
    Whc = nc.declare_dram_parameter("Whc", [P, 2 * 2 * D], BF16, isOutput=False)
    dT = nc.declare_dram_parameter("dT", [P, KC * RPC], BF16, isOutput=False)
    lT = nc.declare_dram_parameter("lT", [P, KC * RPC], BF16, isOutput=False)
    aT = nc.declare_dram_parameter("aT", [P, KC * RPC], FP8, isOutput=False)
    bT = nc.declare_dram_parameter("bT", [P, 2], F32, isOutput=False)
    # transposed output: row = D coordinate (2 halves of 128), col = local row
    out = nc.declare_dram_parameter("out", [2 * P, RPC], BF16, isOutput=True)

    # collective bounce buffers: one gather = one collective of [P, MT*D]
    cc_in = {}
    cc_out = {}
    for g in (1, 2):
        cc_in[g] = nc.dram_tensor(f"cc{g}_in", [P, MT * D], BF16)
        cc_out[g] = nc.dram_tensor(
            f"cc{g}_out", [NCORES * P, MT * D], BF16, addr_space="Shared"
        )

    dT_v = dT.rearrange("p (kc m) -> p kc m", kc=KC)
    lT_v = lT.rearrange("p (kc m) -> p kc m", kc=KC)
    aT_v = aT.rearrange("p (kc m) -> p kc m", kc=KC)
    xT_v = xT.rearrange("p (kc m) -> p kc m", kc=2)
    Whc_v = Whc.rearrange("p (kc m) -> p kc m", kc=2)
    cc_in_v = {k: v.rearrange("p (mt m) -> p mt m", mt=MT) for k, v in cc_in.items()}
    # readback: rank r partition p holds chunks 4r..4r+3 as 4 contiguous D-cols
    cc_out_v = {
        k: v.rearrange("(rc p) (mt m) -> p rc mt m", p=P, mt=MT)
        for k, v in cc_out.items()
    }

    replica_groups = [list(range(NCORES))]

    def allgather(g):
        nc.gpsimd.collective_compute(
            "AllGather",
            mybir.AluOpType.bypass,
            replica_groups=replica_groups,
            ins=[cc_in[g][:]],
            outs=[cc_out[g][:]],
        )

    with tile.TileContext(nc) as tc:
        with (
            tc.tile_pool(name="const", bufs=1) as cpool,
            tc.tile_pool(name="bigmat", bufs=1) as bigpool,
            tc.tile_pool(name="acts", bufs=1) as apool,
            tc.tile_pool(name="psum", bufs=4, space="PSUM") as pspool,
            tc.tile_pool(name="outp", bufs=2) as opool,
        ):
            for _rep in range(repeat):
                if serial and _rep > 0:
                    # full flush between iterations: slope == single-shot latency
                    tc.strict_bb_all_engine_barrier()

                # ---- bulk loads, sync ring, few big DMAs, in need-order ----
                xT_sb = cpool.tile([P, 2, N], BF16, tag="xT")
                Whc_sb = cpool.tile([P, 2, 2 * D], BF16, tag="Whc")
                bT_sb = cpool.tile([P, 2], F32, tag="bT")
                d_sb = bigpool.tile([P, KC, RPC], BF16, tag="d")
                a_sb = bigpool.tile([P, KC, RPC], FP8, tag="a")
                l_sb = bigpool.tile([P, KC, RPC], BF16, tag="l")
                nc.sync.dma_start(out=Whc_sb[:], in_=Whc_v)
                # x in halves so stage A starts after ~1MB lands
                for mh in range(2):
                    s = slice(mh * (N // 2), (mh + 1) * (N // 2))
                    nc.sync.dma_start(out=xT_sb[:, :, s], in_=xT_v[:, :, s])
                if "load" not in ablate:
                    for c in range(2):
                        s = slice(c * (KC // 2), (c + 1) * (KC // 2))
                        nc.sync.dma_start(out=d_sb[:, s, :], in_=dT_v[:, s, :])
                    nc.sync.dma_start(out=a_sb[:], in_=aT_v)
                    nc.sync.dma_start(out=bT_sb[:], in_=bT[:])
                    for c in range(2):
                        s = slice(c * (KC // 2), (c + 1) * (KC // 2))
                        nc.sync.dma_start(out=l_sb[:, s, :], in_=lT_v[:, s, :])
                else:
                    nc.sync.dma_start(out=d_sb[:, :1, :64], in_=dT_v[:, :1, :64])
                    nc.sync.dma_start(out=a_sb[:, :1, :128], in_=aT_v[:, :1, :128])
                    nc.sync.dma_start(out=bT_sb[:], in_=bT[:])
                    nc.sync.dma_start(out=l_sb[:, :1, :64], in_=lT_v[:, :1, :64])

                # ---- stage A (R half): R = relu(x @ aH*W_high), bf16 ----
                R_sb = apool.tile([P, KC, D], BF16, tag="R")
                xw_sb = apool.tile([P, KC, D], FP8, tag="xw")
                if "A" in ablate:
                    nc.sync.dma_start(out=R_sb[:, :1, :64], in_=dT_v[:, :1, :64])
                    nc.sync.dma_start(out=xw_sb[:, :1, :128], in_=aT_v[:, :1, :128])
                if "A" not in ablate:
                    for m in range(KC):
                        psA = pspool.tile([P, D], F32, tag="ps", name=f"psA{m}_{_rep}")
                        for k in range(2):
                            nc.tensor.matmul(
                                out=psA[:],
                                lhsT=xT_sb[:, k, m * P:(m + 1) * P],
                                rhs=Whc_sb[:, k, :D],
                                start=(k == 0),
                                stop=(k == 1),
                            )
                        # alternate relu between Scalar (activation) and Vector
                        # (max with 0) so neither engine's ~0.37us/chunk
                        # serializes the A->B feed
                        if m % 2 == 0:
                            nc.scalar.activation(R_sb[:, m, :], psA[:], RELU)
                        else:
                            nc.vector.tensor_scalar_max(R_sb[:, m, :], psA[:], 0.0)

                def gather_store(g, psts):
                    # PSUM -> SBUF staging on Scalar, per-tile DMA so the first
                    # transfer starts while later tiles are still being copied
                    t = opool.tile([P, MT, D], BF16, tag="gst", name=f"gs{g}_{_rep}")
                    for m in range(MT):
                        # alternate copy engine so the 4 evacuations overlap
                        if m % 2 == 0:
                            nc.scalar.activation(t[:, m, :], psts[m][:], COPY)
                        else:
                            nc.vector.tensor_copy(t[:, m, :], psts[m][:])
                        nc.scalar.dma_start(out=cc_in_v[g][:, m, :], in_=t[:, m, :])
                    return t

                def gather_load(g, half, dst_sb):
                    # first piece small (4 chunks, scalar ring) so the consumer
                    # starts early; the rest rides the idle sync ring in
                    # parallel (bulk loads are done by now)
                    base = 0 if half == 1 else KC // 2
                    rc0 = 0 if half == 1 else NCORES // 2
                    for eng, (c0, c1) in ((nc.scalar, (0, 4)), (nc.sync, (4, 16))):
                        eng.dma_start(
                            out=dst_sb[:, base + c0:base + c1, :].rearrange(
                                "p (rc mt) m -> p rc mt m", mt=MT
                            ),
                            in_=cc_out_v[g][:, rc0 + c0 // MT:rc0 + c1 // MT, :, :],
                        )

                # ---- stage B (chunk-major): P1_loc = d_inv[rows] @ R ----
                if "B" not in ablate:
                    psB = {}
                    for m in range(MT):
                        psB[m] = pspool.tile([P, D], F32, tag="ps", name=f"psB{m}_{_rep}")
                    for c in range(KC):
                        for m in range(MT):
                            nc.tensor.matmul(
                                out=psB[m][:],
                                lhsT=d_sb[:, c, m * P:(m + 1) * P],
                                rhs=R_sb[:, c, :],
                                start=(c == 0),
                                stop=(c == KC - 1),
                            )
                    gather_store(1, psB)
                    if "AG1" not in ablate:
                        allgather(1)

                # ---- stage A (xw half, deferred): xw = fp8(x @ W_conv) ----
                # covers AG1 flight; C chunks 0..15 cover the readback
                if "A" not in ablate:
                    for m in range(KC):
                        psX = pspool.tile([P, D], F32, tag="ps", name=f"psX{m}_{_rep}")
                        for k in range(2):
                            nc.tensor.matmul(
                                out=psX[:],
                                lhsT=xT_sb[:, k, m * P:(m + 1) * P],
                                rhs=Whc_sb[:, k, D:],
                                start=(k == 0),
                                stop=(k == 1),
                            )
                        nc.vector.tensor_copy(xw_sb[:, m, :], psX[:])

                # ---- stage C (transposed): HlT = (A_T.T @ xw).T accumulation
                # psC[h] [P,512] += xw[:,c,h*128:].T @ aT[:,c,:]  (fp8, N=512)
                psC = {}
                if "C" not in ablate:
                    for h in range(2):
                        psC[h] = pspool.tile(
                            [P, RPC], F32, tag="psw", bufs=4, name=f"psC{h}_{_rep}"
                        )

                def stage_c_chunks(cs, first, last, lhs=None, off=0):
                    src = xw_sb if lhs is None else lhs
                    for c in cs:
                        for h in range(2):
                            nc.tensor.matmul(
                                out=psC[h][:],
                                lhsT=src[:, c - off, h * P:(h + 1) * P],
                                rhs=a_sb[:, c, :],
                                start=(first and c == cs[0]),
                                stop=(last and c == cs[-1]),
                            )

                if "C" not in ablate:
                    stage_c_chunks(list(range(16)), True, False)

                # ---- stage D: P2_loc = lap[rows] @ P1, split on rb halves ----
                P1_sb = apool.tile([P, KC, D], BF16, tag="P1")
                gather_load(1, 1, P1_sb)
                gather_load(1, 2, P1_sb)
                psD = {}
                if "D" not in ablate:
                    for m in range(MT):
                        psD[m] = pspool.tile([P, D], F32, tag="ps", name=f"psD{m}_{_rep}")
                    # chunk-block-major: consume each readback piece for all m
                    # before needing the next piece
                    blocks = [HALF1[:4], HALF1[4:], HALF2[:4], HALF2[4:]]
                    for bi, blk in enumerate(blocks):
                        for m in range(MT):
                            for c in blk:
                                nc.tensor.matmul(
                                    out=psD[m][:],
                                    lhsT=l_sb[:, c, m * P:(m + 1) * P],
                                    rhs=P1_sb[:, c, :],
                                    start=(bi == 0 and c == blk[0]),
                                    stop=(bi == len(blocks) - 1 and c == blk[-1]),
                                )
                    t2 = gather_store(2, psD)
                    if "AG2" not in ablate:
                        allgather(2)

                # ---- stage C (rest): covers AG2 flight; fold bias on evacuate
                # fence: xw2 = xw[16:] + 0, where the 0 is derived from the AG2
                # staging tile -- a data dependency that stops Tile from
                # hoisting these matmuls ahead of the AG2 doorbell (v5 ran all
                # of C early, leaving the AG2 flight uncovered and E cold)
                HlT_sb = opool.tile([P, 2, RPC], BF16, tag="HlT")
                if "C" not in ablate and "D" not in ablate:
                    fence_t = opool.tile([P, 1], F32, tag="fence")
                    nc.vector.tensor_scalar_mul(fence_t[:], t2[:, MT - 1, :1], 0.0)
                    xw2_sb = apool.tile([P, KC // 2, D], FP8, tag="xw2")
                    nc.vector.tensor_scalar_add(
                        xw2_sb[:], xw_sb[:, KC // 2:, :], fence_t[:]
                    )
                if "C" not in ablate:
                    fl = xw2_sb if "D" not in ablate else None
                    stage_c_chunks(list(range(16, KC)), False, True,
                                   lhs=fl, off=0 if fl is None else 16)
                    for h in range(2):
                        nc.scalar.activation(
                            HlT_sb[:, h, :], psC[h][:], IDENT, bias=bT_sb[:, h:h + 1]
                        )
                else:
                    for h in range(2):
                        nc.vector.memset(HlT_sb[:, h, :], 0.0)

                # ---- stage E (transposed): HhT[h] += P2[:,c,h].T @ dT[:,c,:]
                P2_sb = apool.tile([P, KC, D], BF16, tag="P2")
                gather_load(2, 1, P2_sb)
                gather_load(2, 2, P2_sb)
                if "E" not in ablate:
                    psE = {}
                    for h in range(2):
                        psE[h] = pspool.tile(
                            [P, RPC], F32, tag="psw", bufs=4, name=f"psE{h}_{_rep}"
                        )
                    blocks = [HALF1[:4], HALF1[4:], HALF2[:4], HALF2[4:]]
                    for bi, blk in enumerate(blocks):
                        for c in blk:
                            for h in range(2):
                                nc.tensor.matmul(
                                    out=psE[h][:],
                                    lhsT=P2_sb[:, c, h * P:(h + 1) * P],
                                    rhs=d_sb[:, c, :],
                                    start=(bi == 0 and c == blk[0]),
                                    stop=(bi == len(blocks) - 1 and c == blk[-1]),
                                )
                    for h in range(2):
                        o_sb = opool.tile([P, RPC], BF16, tag="osb", name=f"os{h}_{_rep}")
                        nc.vector.tensor_add(o_sb[:], psE[h][:], HlT_sb[:, h, :])
                        nc.scalar.dma_start(out=out[h * P:(h + 1) * P, :], in_=o_sb[:])

    nc.finalize()
    return nc


def prep_inputs(x, edge_index, lap, d_inv, W_high, W_conv, b_conv, aL, aH):
    """Host-side sharding/layout: build per-core input maps."""
    x = np.asarray(x, dtype=np.float32)
    lap = np.asarray(lap, dtype=np.float32)
    d_inv = np.asarray(d_inv, dtype=np.float32)
    W_high = np.asarray(W_high, dtype=np.float32)
    W_conv = np.asarray(W_conv, dtype=np.float32)
    b_conv = np.asarray(b_conv, dtype=np.float32)
    aLs = float(np.asarray(aL).reshape(-1)[0])
    aHs = float(np.asarray(aH).reshape(-1)[0])
    src = np.asarray(edge_index[0], dtype=np.int64)
    dst = np.asarray(edge_index[1], dtype=np.int64)

    # symmetric GCN normalization (with self-loops) folded into a dense adjacency
    deg = np.bincount(dst, minlength=N).astype(np.float32) + 1.0
    dis = 1.0 / np.sqrt(deg)
    A_T = np.zeros((N, N), dtype=np.float32)           # A_T[src, dst]
    np.add.at(A_T, (src, dst), aLs * dis[src] * dis[dst])
    A_T[np.arange(N), np.arange(N)] += aLs * dis * dis

    def to_pkm(arrT):
        # [K, M] -> [P, kc*M]: element (p, c*M + m) = arrT[128*c + p, m]
        Kdim, Mdim = arrT.shape
        kc = Kdim // P
        a = arrT.reshape(kc, P, Mdim)
        return np.ascontiguousarray(a.transpose(1, 0, 2).reshape(P, kc * Mdim))

    xT = to_pkm(np.ascontiguousarray(x.T).astype(nbf16))
    Whc = to_pkm(np.concatenate([W_high * aHs, W_conv], axis=1).astype(nbf16))
    # bias along D (partition axis in transposed space): [P, 2] f32
    bTm = np.ascontiguousarray((aLs * b_conv).reshape(2, P).T).astype(np.float32)
    dT_full = np.ascontiguousarray(d_inv.T).astype(nbf16)
    lT_full = np.ascontiguousarray(lap.T).astype(nbf16)
    aT_full = np.clip(A_T, -240, 240).astype(nfp8)

    in_maps = []
    for i in range(NCORES):
        sl = slice(i * RPC, (i + 1) * RPC)
        in_maps.append({
            "xT": xT,
            "Whc": Whc,
            "dT": to_pkm(dT_full[:, sl]),
            "lT": to_pkm(lT_full[:, sl]),
            "aT": to_pkm(aT_full[:, sl]),
            "bT": bTm,
        })
    return in_maps


def kernel(x, edge_index, lap, d_inv, W_high, W_conv, b_conv, aL, aH):
    in_maps = prep_inputs(x, edge_index, lap, d_inv, W_high, W_conv, b_conv, aL, aH)
    nc = build_program()
    res = run_bass_kernel_spmd(nc, in_maps, list(range(NCORES)))
    # per-core output is D-major [256, 512]; transpose back to [512, 256]
    return np.concatenate(
        [res.results[i]["out"].T for i in range(NCORES)], axis=0
    ).astype(np.float32)


# revision 20
# speedup vs baseline: 1.3813x; 1.2072x over previous
"""FBGCN layer on 8 Trainium2 NeuronCores — v5.

Math (reference):
    Lhp = (d_inv @ lap) @ d_inv
    Hh  = Lhp @ relu(x @ W_high)
    Hl  = GCNConv(x, edge_index, W_conv, b_conv)
    out = aL * Hl + aH * Hh

v5 vs v3/v4 (trace-driven):
  * Collectives are ~15-25us nearly size-independent here, so v4's
    split-gather regressed (4 collectives > 2).  Back to exactly TWO
    AllGathers (structural minimum for the 3-matmul chain).
  * Staging copies on Scalar (v3 queued them behind 20+ fp8 CASTs on
    the DVE FIFO, delaying the AG1 doorbell ~10us), into ONE [P,MT*D]
    staging tile -> ONE 512KB DMA to the bounce buffer (fewer trigger
    slots, bigger transfer).
  * Stages C and E run in transposed form (out.T = V.T @ M.T): the thin
    operand becomes lhsT (natural layout, as delivered by the AG
    readback) and the wide matrix (aT / dT, host-pre-transposed) is the
    rhs with free dim 512 -> 64 matmuls instead of 128 per stage at
    ~98% PE column efficiency.  Output lands D-major [2*P, RPC]; the
    host transposes (free).  The C/E adds become per-partition bias
    (bT) + tensor_tensor in transposed space.
  * C is split into chunk ranges placed to plug both AG windows:
    chunks 0..15 after A_xw (AG1 flight), 16..31 after the AG2 doorbell.
  * Bulk loads in 8 large DMAs, need-order; out stores on scalar ring.
"""

import numpy as np
import ml_dtypes

import concourse.bass as bass
import concourse.mybir as mybir
import concourse.tile as tile
from concourse import bacc
from concourse.bass_utils import run_bass_kernel_spmd

N = 4096
D = 256
E = 131072
NCORES = 8
RPC = N // NCORES          # rows per core = 512
KC = N // 128              # contraction chunks = 32
MT = RPC // 128            # output row tiles per core = 4
P = 128

BF16 = mybir.dt.bfloat16
F32 = mybir.dt.float32
FP8 = mybir.dt.float8e4
nbf16 = ml_dtypes.bfloat16
nfp8 = ml_dtypes.float8_e4m3

RELU = mybir.ActivationFunctionType.Relu
COPY = mybir.ActivationFunctionType.Copy
IDENT = mybir.ActivationFunctionType.Identity

# readback halves: slots (= global chunks) 0..15 are ranks 0-3, 16..31 ranks 4-7
HALF1 = list(range(KC // 2))
HALF2 = list(range(KC // 2, KC))


def build_program(repeat: int = 1, ablate: frozenset = frozenset(), serial: bool = True):
    """Build the SPMD per-core program (identical on all cores)."""
    nc = bacc.Bacc(num_devices=NCORES)

    # ---- I/O ----  (matrix inputs come host-pre-transposed to [P, kc*m])
    xT = nc.declare_dram_parameter("xT", [P, 2 * N], BF16, isOutput=False)
    Whc = nc.declare_dram_parameter("Whc", [P, 2 * 2 * D], BF16, isOutput=False)
    dT = nc.declare_dram_parameter("dT", [P, KC * RPC], BF16, isOutput=False)
    lT = nc.declare_dram_parameter("lT", [P, KC * RPC], BF16, isOutput=False)
    aT = nc.declare_dram_parameter("aT", [P, KC * RPC], FP8, isOutput=False)
    bT = nc.declare_dram_parameter("bT", [P, 2], F32, isOutput=False)
    # transposed output: row = D coordinate (2 halves of 128), col = local row
    out = nc.declare_dram_parameter("out", [2 * P, RPC], BF16, isOutput=True)

    # collective bounce buffers: one gather = one collective of [P, MT*D]
    cc_in = {}
    cc_out = {}
    for g in (1, 2):
        cc_in[g] = nc.dram_tensor(f"cc{g}_in", [P, MT * D], BF16)
        cc_out[g] = nc.dram_tensor(
            f"cc{g}_out", [NCORES * P, MT * D], BF16, addr_space="Shared"
        )

    dT_v = dT.rearrange("p (kc m) -> p kc m", kc=KC)
    lT_v = lT.rearrange("p (kc m) -> p kc m", kc=KC)
    aT_v = aT.rearrange("p (kc m) -> p kc m", kc=KC)
    xT_v = xT.rearrange("p (kc m) -> p kc m", kc=2)
    Whc_v = Whc.rearrange("p (kc m) -> p kc m", kc=2)
    cc_in_v = {k: v.rearrange("p (mt m) -> p mt m", mt=MT) for k, v in cc_in.items()}
    # readback: rank r partition p holds chunks 4r..4r+3 as 4 contiguous D-cols
    cc_out_v = {
        k: v.rearrange("(rc p) (mt m) -> p rc mt m", p=P, mt=MT)
        for k, v in cc_out.items()
    }

    replica_groups = [list(range(NCORES))]

    def allgather(g):
        nc.gpsimd.collective_compute(
            "AllGather",
            mybir.AluOpType.bypass,
            replica_groups=replica_groups,
            ins=[cc_in[g][:]],
            outs=[cc_out[g][:]],
        )

    with tile.TileContext(nc) as tc:
        with (
            tc.tile_pool(name="const", bufs=1) as cpool,
            tc.tile_pool(name="bigmat", bufs=1) as bigpool,
            tc.tile_pool(name="acts", bufs=1) as apool,
            tc.tile_pool(name="psum", bufs=4, space="PSUM") as pspool,
            tc.tile_pool(name="outp", bufs=2) as opool,
        ):
            for _rep in range(repeat):
                if serial and _rep > 0:
                    # full flush between iterations: slope == single-shot latency
                    tc.strict_bb_all_engine_barrier()

                # ---- bulk loads, sync ring, few big DMAs, in need-order ----
                xT_sb = cpool.tile([P, 2, N], BF16, tag="xT")
                Whc_sb = cpool.tile([P, 2, 2 * D], BF16, tag="Whc")
                bT_sb = cpool.tile([P, 2], F32, tag="bT")
                d_sb = bigpool.tile([P, KC, RPC], BF16, tag="d")
                a_sb = bigpool.tile([P, KC, RPC], FP8, tag="a")
                l_sb = bigpool.tile([P, KC, RPC], BF16, tag="l")
                nc.sync.dma_start(out=Whc_sb[:], in_=Whc_v)
                # x in halves so stage A starts after ~1MB lands
                for mh in range(2):
                    s = slice(mh * (N // 2), (mh + 1) * (N // 2))
                    nc.sync.dma_start(out=xT_sb[:, :, s], in_=xT_v[:, :, s])
                if "load" not in ablate:
                    for c in range(2):
                        s = slice(c * (KC // 2), (c + 1) * (KC // 2))
                        nc.sync.dma_start(out=d_sb[:, s, :], in_=dT_v[:, s, :])
                    nc.sync.dma_start(out=a_sb[:], in_=aT_v)
                    nc.sync.dma_start(out=bT_sb[:], in_=bT[:])
                    for c in range(2):
                        s = slice(c * (KC // 2), (c + 1) * (KC // 2))
                        nc.sync.dma_start(out=l_sb[:, s, :], in_=lT_v[:, s, :])
                else:
                    nc.sync.dma_start(out=d_sb[:, :1, :64], in_=dT_v[:, :1, :64])
                    nc.sync.dma_start(out=a_sb[:, :1, :128], in_=aT_v[:, :1, :128])
                    nc.sync.dma_start(out=bT_sb[:], in_=bT[:])
                    nc.sync.dma_start(out=l_sb[:, :1, :64], in_=lT_v[:, :1, :64])

                # ---- stage A (R half): R = relu(x @ aH*W_high), bf16 ----
                R_sb = apool.tile([P, KC, D], BF16, tag="R")
                xw_sb = apool.tile([P, KC, D], FP8, tag="xw")
                if "A" in ablate:
                    nc.sync.dma_start(out=R_sb[:, :1, :64], in_=dT_v[:, :1, :64])
                    nc.sync.dma_start(out=xw_sb[:, :1, :128], in_=aT_v[:, :1, :128])
                if "A" not in ablate:
                    for m in range(KC):
                        psA = pspool.tile([P, D], F32, tag="ps", name=f"psA{m}_{_rep}")
                        for k in range(2):
                            nc.tensor.matmul(
                                out=psA[:],
                                lhsT=xT_sb[:, k, m * P:(m + 1) * P],
                                rhs=Whc_sb[:, k, :D],
                                start=(k == 0),
                                stop=(k == 1),
                            )
                        # alternate relu between Scalar (activation) and Vector
                        # (max with 0) so neither engine's ~0.37us/chunk
                        # serializes the A->B feed
                        if m % 2 == 0:
                            nc.scalar.activation(R_sb[:, m, :], psA[:], RELU)
                        else:
                            nc.vector.tensor_scalar_max(R_sb[:, m, :], psA[:], 0.0)

                def gather_store(g, psts):
                    # PSUM -> SBUF staging on Scalar, per-tile DMA so the first
                    # transfer starts while later tiles are still being copied
                    t = opool.tile([P, MT, D], BF16, tag="gst", name=f"gs{g}_{_rep}")
                    for m in range(MT):
                        # alternate copy engine so the 4 evacuations overlap
                        if m % 2 == 0:
                            nc.scalar.activation(t[:, m, :], psts[m][:], COPY)
                        else:
                            nc.vector.tensor_copy(t[:, m, :], psts[m][:])
                        nc.scalar.dma_start(out=cc_in_v[g][:, m, :], in_=t[:, m, :])
                    return t

                def gather_load(g, half, dst_sb):
                    # first piece small (4 chunks, scalar ring) so the consumer
                    # starts early; the rest rides the idle sync ring in
                    # parallel (bulk loads are done by now)
                    base = 0 if half == 1 else KC // 2
                    rc0 = 0 if half == 1 else NCORES // 2
                    for eng, (c0, c1) in ((nc.scalar, (0, 4)), (nc.sync, (4, 16))):
                        eng.dma_start(
                            out=dst_sb[:, base + c0:base + c1, :].rearrange(
                                "p (rc mt) m -> p rc mt m", mt=MT
                            ),
                            in_=cc_out_v[g][:, rc0 + c0 // MT:rc0 + c1 // MT, :, :],
                        )

                # ---- stage B (chunk-major): P1_loc = d_inv[rows] @ R ----
                if "B" not in ablate:
                    psB = {}
                    t1s = {}
                    for m in range(MT):
                        psB[m] = pspool.tile([P, D], F32, tag="ps", name=f"psB{m}_{_rep}")
                    # m-pair-major: pair {0,1} finishes at B's midpoint, so its
                    # staging copies+DMAs overlap pair {2,3}'s matmuls
                    t1 = opool.tile([P, MT, D], BF16, tag="gst", name=f"gs1_{_rep}")
                    for pair in (0, 1):
                        for c in range(KC):
                            for m in (2 * pair, 2 * pair + 1):
                                nc.tensor.matmul(
                                    out=psB[m][:],
                                    lhsT=d_sb[:, c, m * P:(m + 1) * P],
                                    rhs=R_sb[:, c, :],
                                    start=(c == 0),
                                    stop=(c == KC - 1),
                                )
                        for m in (2 * pair, 2 * pair + 1):
                            if m % 2 == 0:
                                nc.scalar.activation(t1[:, m, :], psB[m][:], COPY)
                            else:
                                nc.vector.tensor_copy(t1[:, m, :], psB[m][:])
                            nc.scalar.dma_start(
                                out=cc_in_v[1][:, m, :], in_=t1[:, m, :]
                            )
                    if "AG1" not in ablate:
                        allgather(1)

                # ---- stage A (xw half, deferred): xw = fp8(x @ W_conv) ----
                # covers AG1 flight; C chunks 0..15 cover the readback
                if "A" not in ablate:
                    for m in range(KC):
                        psX = pspool.tile([P, D], F32, tag="ps", name=f"psX{m}_{_rep}")
                        for k in range(2):
                            nc.tensor.matmul(
                                out=psX[:],
                                lhsT=xT_sb[:, k, m * P:(m + 1) * P],
                                rhs=Whc_sb[:, k, D:],
                                start=(k == 0),
                                stop=(k == 1),
                            )
                        nc.vector.tensor_copy(xw_sb[:, m, :], psX[:])

                # ---- stage C (transposed): HlT = (A_T.T @ xw).T accumulation
                # psC[h] [P,512] += xw[:,c,h*128:].T @ aT[:,c,:]  (fp8, N=512)
                psC = {}
                if "C" not in ablate:
                    for h in range(2):
                        psC[h] = pspool.tile(
                            [P, RPC], F32, tag="psw", bufs=4, name=f"psC{h}_{_rep}"
                        )

                def stage_c_chunks(cs, first, last, lhs=None, off=0):
                    src = xw_sb if lhs is None else lhs
                    for c in cs:
                        for h in range(2):
                            nc.tensor.matmul(
                                out=psC[h][:],
                                lhsT=src[:, c - off, h * P:(h + 1) * P],
                                rhs=a_sb[:, c, :],
                                start=(first and c == cs[0]),
                                stop=(last and c == cs[-1]),
                            )

                if "C" not in ablate:
                    stage_c_chunks(list(range(13)), True, False)

                # ---- stage D: P2_loc = lap[rows] @ P1, split on rb halves ----
                P1_sb = apool.tile([P, KC, D], BF16, tag="P1")
                gather_load(1, 1, P1_sb)
                gather_load(1, 2, P1_sb)
                psD = {}
                if "D" not in ablate:
                    for m in range(MT):
                        psD[m] = pspool.tile([P, D], F32, tag="ps", name=f"psD{m}_{_rep}")
                    # chunk-block-major: consume each readback piece for all m
                    # before needing the next piece
                    blocks = [HALF1[:4], HALF1[4:], HALF2[:4], HALF2[4:]]
                    for bi, blk in enumerate(blocks):
                        for m in range(MT):
                            for c in blk:
                                nc.tensor.matmul(
                                    out=psD[m][:],
                                    lhsT=l_sb[:, c, m * P:(m + 1) * P],
                                    rhs=P1_sb[:, c, :],
                                    start=(bi == 0 and c == blk[0]),
                                    stop=(bi == len(blocks) - 1 and c == blk[-1]),
                                )
                    t2 = gather_store(2, psD)
                    if "AG2" not in ablate:
                        allgather(2)

                # ---- stage C (rest): covers AG2 flight; fold bias on evacuate
                # fence: xw2 = xw[16:] + 0, where the 0 is derived from the AG2
                # staging tile -- a data dependency that stops Tile from
                # hoisting these matmuls ahead of the AG2 doorbell (v5 ran all
                # of C early, leaving the AG2 flight uncovered and E cold)
                HlT_sb = opool.tile([P, 2, RPC], BF16, tag="HlT")
                CSPLIT = 13
                if "C" not in ablate and "D" not in ablate:
                    fence_t = opool.tile([P, 1], F32, tag="fence")
                    nc.vector.tensor_scalar_mul(fence_t[:], t2[:, MT - 1, :1], 0.0)
                    xw2_sb = apool.tile([P, KC - CSPLIT, D], FP8, tag="xw2")
                    nc.vector.tensor_scalar_add(
                        xw2_sb[:], xw_sb[:, CSPLIT:, :], fence_t[:]
                    )
                if "C" not in ablate:
                    fl = xw2_sb if "D" not in ablate else None
                    stage_c_chunks(list(range(CSPLIT, KC)), False, True,
                                   lhs=fl, off=0 if fl is None else CSPLIT)
                    for h in range(2):
                        nc.scalar.activation(
                            HlT_sb[:, h, :], psC[h][:], IDENT, bias=bT_sb[:, h:h + 1]
                        )
                else:
                    for h in range(2):
                        nc.vector.memset(HlT_sb[:, h, :], 0.0)

                # ---- stage E (transposed): HhT[h] += P2[:,c,h].T @ dT[:,c,:]
                P2_sb = apool.tile([P, KC, D], BF16, tag="P2")
                gather_load(2, 1, P2_sb)
                gather_load(2, 2, P2_sb)
                if "E" not in ablate:
                    psE = {}
                    for h in range(2):
                        psE[h] = pspool.tile(
                            [P, RPC], F32, tag="psw", bufs=4, name=f"psE{h}_{_rep}"
                        )
                    blocks = [HALF1[:4], HALF1[4:], HALF2[:4], HALF2[4:]]
                    for bi, blk in enumerate(blocks):
                        for c in blk:
                            for h in range(2):
                                nc.tensor.matmul(
                                    out=psE[h][:],
                                    lhsT=P2_sb[:, c, h * P:(h + 1) * P],
                                    rhs=d_sb[:, c, :],
                                    start=(bi == 0 and c == blk[0]),
                                    stop=(bi == len(blocks) - 1 and c == blk[-1]),
                                )
                    for h in range(2):
                        o_sb = opool.tile([P, RPC], BF16, tag="osb", name=f"os{h}_{_rep}")
                        nc.vector.tensor_add(o_sb[:], psE[h][:], HlT_sb[:, h, :])
                        nc.scalar.dma_start(out=out[h * P:(h + 1) * P, :], in_=o_sb[:])

    nc.finalize()
    return nc


def prep_inputs(x, edge_index, lap, d_inv, W_high, W_conv, b_conv, aL, aH):
    """Host-side sharding/layout: build per-core input maps."""
    x = np.asarray(x, dtype=np.float32)
    lap = np.asarray(lap, dtype=np.float32)
    d_inv = np.asarray(d_inv, dtype=np.float32)
    W_high = np.asarray(W_high, dtype=np.float32)
    W_conv = np.asarray(W_conv, dtype=np.float32)
    b_conv = np.asarray(b_conv, dtype=np.float32)
    aLs = float(np.asarray(aL).reshape(-1)[0])
    aHs = float(np.asarray(aH).reshape(-1)[0])
    src = np.asarray(edge_index[0], dtype=np.int64)
    dst = np.asarray(edge_index[1], dtype=np.int64)

    # symmetric GCN normalization (with self-loops) folded into a dense adjacency
    deg = np.bincount(dst, minlength=N).astype(np.float32) + 1.0
    dis = 1.0 / np.sqrt(deg)
    A_T = np.zeros((N, N), dtype=np.float32)           # A_T[src, dst]
    np.add.at(A_T, (src, dst), aLs * dis[src] * dis[dst])
    A_T[np.arange(N), np.arange(N)] += aLs * dis * dis

    def to_pkm(arrT):
        # [K, M] -> [P, kc*M]: element (p, c*M + m) = arrT[128*c + p, m]
        Kdim, Mdim = arrT.shape
        kc = Kdim // P
        a = arrT.reshape(kc, P, Mdim)
        return np.ascontiguousarray(a.transpose(1, 0, 2).reshape(P, kc * Mdim))

    xT = to_pkm(np.ascontiguousarray(x.T).astype(nbf16))
    Whc = to_pkm(np.concatenate([W_high * aHs, W_conv], axis=1).astype(nbf16))
    # bias along D (partition axis in transposed space): [P, 2] f32
    bTm = np.ascontiguousarray((aLs * b_conv).reshape(2, P).T).astype(np.float32)
    dT_full = np.ascontiguousarray(d_inv.T).astype(nbf16)
    lT_full = np.ascontiguousarray(lap.T).astype(nbf16)
    aT_full = np.clip(A_T, -240, 240).astype(nfp8)

    in_maps = []
    for i in range(NCORES):
        sl = slice(i * RPC, (i + 1) * RPC)
        in_maps.append({
            "xT": xT,
            "Whc": Whc,
            "dT": to_pkm(dT_full[:, sl]),
            "lT": to_pkm(lT_full[:, sl]),
            "aT": to_pkm(aT_full[:, sl]),
            "bT": bTm,
        })
    return in_maps


def kernel(x, edge_index, lap, d_inv, W_high, W_conv, b_conv, aL, aH):
    in_maps = prep_inputs(x, edge_index, lap, d_inv, W_high, W_conv, b_conv, aL, aH)
    nc = build_program()
    res = run_bass_kernel_spmd(nc, in_maps, list(range(NCORES)))
    # per-core output is D-major [256, 512]; transpose back to [512, 256]
    return np.concatenate(
        [res.results[i]["out"].T for i in range(NCORES)], axis=0
    ).astype(np.float32)
